# revision 21
# baseline (speedup 1.0000x reference)
"""FFM (field-aware factorization machine) forward kernel for 8 TRN2 NeuronCores.

y[b] = x[b] @ w_lin + b_lin + sum_{i<j} Wu[i,j] x[b,i] x[b,j]
with Wu = triu(Wmat, 1), Wmat[i,j] = <v[i, field[j]], v[j, field[i]]>.

Strategy:
  - Host: build Wmat from (v, field_idx)  [tiny: 256x256x8], symmetrize
    S = (Wu + Wu^T)/2, eigendecompose S = Q diag(lam) Q^T.  Then
    x^T Wu x = sum_n lam_n (x . q_n)^2.
  - Device (data-parallel over batch, 8 cores): per core, z^T = Q^T X^T via
    float32r matmuls (full PE rate, ~tf32 input rounding, fp32 accumulate),
    squares split across ScalarE (direct from PSUM) and VectorE (copy +
    square), then a lambda-weighted partition-reduction matmul whose [1, F]
    outputs land on PSUM partition rows 0/32/64/96 so four chunks copy out
    as one [4, F] tile.
  - Host feeds x pre-transposed ([256, B/8] per core) so the contraction
    dim lands on SBUF partitions with zero on-device transposes.
"""

import numpy as np

_B, _N = 65536, 256
_NCORES = 8
_BS = _B // _NCORES  # 8192 batch rows per core
_FCH = 512           # matmul moving free-dim chunk (1 PSUM bank of fp32)
_DCH = 2048          # DMA chunk columns (1 MiB per half-slab)
_NCH = _BS // _FCH   # 16 chunks per core
# chunks whose pz1 square runs directly on ScalarE (load balance ACT vs DVE)
_ACT_EXTRA = {0, 3, 5, 8, 11, 13}

_compiled_nc = {}


def _round_fp32r(a):
    """Round-to-nearest fp32 -> fp32r (tf32-like, 11 mantissa bits)."""
    from neuron_dtypes import static_cast_fp32_to_fp32r

    a = np.ascontiguousarray(a, dtype=np.float32)
    r = static_cast_fp32_to_fp32r(a)
    return np.asarray(r).view(np.uint32).view(np.float32).reshape(a.shape)


def _build_nc(reps=1):
    from concourse import bacc, mybir, tile

    f32 = mybir.dt.float32
    f32r = mybir.dt.float32r
    Act = mybir.ActivationFunctionType
    Alu = mybir.AluOpType

    nc = bacc.Bacc("TRN2", target_bir_lowering=False, debug=False)

    xt = nc.dram_tensor("xt", [_N, _BS], f32r, kind="ExternalInput").ap()
    q = nc.dram_tensor("q", [_N, _N], f32r, kind="ExternalInput").ap()
    # masked lambda table: lam[p, m*8 + ch*4 + j] = lam[ch*128+p] * (j == m)
    lam = nc.dram_tensor("lam", [128, 32], f32r, kind="ExternalInput").ap()
    # y_dram[m, g*512 + f] = y_quad[core_base + 2048*g + 512*m + f]
    y = nc.dram_tensor("y", [4, _BS // 4], f32, kind="ExternalOutput").ap()

    with tile.TileContext(nc) as tc:
        with (
            tc.tile_pool(name="const", bufs=1) as cpool,
            tc.tile_pool(name="xin", bufs=3) as xpool,
            tc.tile_pool(name="zsq", bufs=3) as zpool,
            tc.tile_pool(name="yout", bufs=1) as ypool,
            tc.tile_pool(name="pz", bufs=2, space="PSUM") as pzpool,
            tc.tile_pool(name="py", bufs=2, space="PSUM") as pypool,
        ):
            # Constants: Q split into two 128-row chunks; lam packed [128, 2].
            q0 = cpool.tile([128, _N], f32r)
            q1 = cpool.tile([128, _N], f32r)
            nc.sync.dma_start(q0[:], q[0:128, :])
            nc.sync.dma_start(q1[:], q[128:256, :])
            lam_sb = cpool.tile([128, 32], f32r)
            nc.sync.dma_start(lam_sb[:], lam[:, :])

            y_sb = ypool.tile([4, (_NCH // 4) * _FCH], f32)

            n_dma = _BS // _DCH
            k_per = _DCH // _FCH

            state = {"py_t": None}

            def emit_reduce(prev):
                c, zs0, zs1 = prev
                m = c % 4
                if m == 0:
                    state["py_t"] = pypool.tile([4, _FCH], f32, tag="py",
                                                name="py_t")
                py_t = state["py_t"]
                # lhsT col j = lam_chunk * (j == m): chunk lands on row m,
                # rows != m accumulate zeros.
                nc.tensor.matmul(py_t[:], lam_sb[:, m * 8:m * 8 + 4], zs0[:],
                                 start=(m == 0), stop=False)
                nc.tensor.matmul(py_t[:], lam_sb[:, m * 8 + 4:m * 8 + 8],
                                 zs1[:], start=False, stop=(m == 3))
                if m == 3:
                    g = c // 4
                    nc.vector.tensor_copy(
                        y_sb[:, g * _FCH:(g + 1) * _FCH], py_t[:])

            prev = None
            for _rep in range(reps):
              for d in range(n_dma):
                x0 = xpool.tile([128, _DCH], f32r, tag="x0")
                x1 = xpool.tile([128, _DCH], f32r, tag="x1")
                nc.sync.dma_start(x0[:], xt[0:128, d * _DCH:(d + 1) * _DCH])
                nc.sync.dma_start(x1[:], xt[128:256, d * _DCH:(d + 1) * _DCH])
                for k in range(k_per):
                    c = d * k_per + k
                    r0 = x0[:, k * _FCH:(k + 1) * _FCH]
                    r1 = x1[:, k * _FCH:(k + 1) * _FCH]
                    pz0 = pzpool.tile([128, _FCH], f32, tag="pz0")
                    pz1 = pzpool.tile([128, _FCH], f32, tag="pz1")
                    # z^T[n, b] = sum_i Q[i, n] * xT[i, b], n-chunk 0 and 1
                    nc.tensor.matmul(pz0[:], q0[:, 0:128], r0,
                                     start=True, stop=False)
                    nc.tensor.matmul(pz0[:], q1[:, 0:128], r1,
                                     start=False, stop=True)
                    nc.tensor.matmul(pz1[:], q0[:, 128:256], r0,
                                     start=True, stop=False)
                    nc.tensor.matmul(pz1[:], q1[:, 128:256], r1,
                                     start=False, stop=True)
                    # Reduce the previous chunk while this one's squares cook:
                    # keeps PE fed (zsq of c-1 is ready by the time the 4
                    # z-matmuls of chunk c drain).
                    if prev is not None:
                        emit_reduce(prev)
                    zs0 = zpool.tile([128, _FCH], f32r, tag="zs0")
                    zs1 = zpool.tile([128, _FCH], f32r, tag="zs1")
                    nc.scalar.activation(zs0[:], pz0[:], Act.Square)
                    if c in _ACT_EXTRA:
                        nc.scalar.activation(zs1[:], pz1[:], Act.Square)
                    else:
                        t1 = zpool.tile([128, _FCH], f32, tag="t1")
                        nc.vector.tensor_copy(t1[:], pz1[:])
                        nc.vector.tensor_mul(zs1[:], t1[:], t1[:])
                    prev = (c, zs0, zs1)
            emit_reduce(prev)

            nc.sync.dma_start(y[:, :], y_sb[:])

    nc.compile()
    return nc


def _get_nc(reps=1):
    if reps not in _compiled_nc:
        _compiled_nc[reps] = _build_nc(reps)
    return _compiled_nc[reps]


def _host_prep(x, w_lin, b_lin, v, field_idx):
    """Host-side tiny-param preprocessing + sharding. Returns (in_maps, lin)."""
    x = np.asarray(x, dtype=np.float32)
    w_lin = np.asarray(w_lin, dtype=np.float32)
    b_lin = np.asarray(b_lin, dtype=np.float32)
    v = np.asarray(v, dtype=np.float64)
    field_idx = np.asarray(field_idx, dtype=np.int64)

    # Wmat[i, j] = <v[i, field[j]], v[j, field[i]]>
    A = v[:, field_idx, :]                       # [N, N, K]
    Wmat = np.einsum('ijk,jik->ij', A, A)        # [N, N]
    Wu = np.triu(Wmat, 1)
    S = (Wu + Wu.T) * 0.5
    lam, Q = np.linalg.eigh(S)                   # S = Q diag(lam) Q^T
    Q32 = _round_fp32r(Q.astype(np.float32))
    # masked lambda table [128, 32]: col m*8 + ch*4 + j = lam[ch*128+p]*(j==m)
    lam2 = lam.astype(np.float32).reshape(2, 128).T  # [p, ch]
    lam_tbl = np.zeros((128, 4, 2, 4), dtype=np.float32)
    for m in range(4):
        lam_tbl[:, m, :, m] = lam2
    lam32 = _round_fp32r(lam_tbl.reshape(128, 32))

    # x transposed + sharded along batch, pre-rounded to fp32r
    xts = x.reshape(_NCORES, _BS, _N).transpose(0, 2, 1)  # [8, N, BS]
    xts = _round_fp32r(np.ascontiguousarray(xts))

    in_maps = [
        {"xt": xts[i], "q": Q32, "lam": lam32} for i in range(_NCORES)
    ]
    lin = x @ w_lin + b_lin[0]                   # linear part on host (0.4% of FLOPs)
    return in_maps, lin


def _unscramble(y_core):
    """[4, 2048] device layout -> [8192] batch order."""
    return y_core.reshape(4, 4, _FCH).transpose(1, 0, 2).reshape(_BS)


def _run_device(in_maps, trace=False, reps=1):
    from concourse.bass_utils import run_bass_kernel_spmd

    nc = _get_nc(reps)
    res = run_bass_kernel_spmd(
        nc, in_maps, core_ids=list(range(_NCORES)), trace=trace
    )
    yq = np.concatenate(
        [_unscramble(res.results[i]["y"]) for i in range(_NCORES)]
    )
    return yq, res


def kernel(x, w_lin, b_lin, v, field_idx):
    in_maps, lin = _host_prep(x, w_lin, b_lin, v, field_idx)
    yq, _ = _run_device(in_maps, trace=False)
    return (lin + yq).astype(np.float32)[:, None]
